# revision 28
# baseline (speedup 1.0000x reference)
"""Causal MHSA kernel for Trainium2, 8 NeuronCores, tensor-parallel over heads.

Problem (hardcoded): x[2, 2048, 1024], w_in[3072, 1024], b_in[3072],
w_out[1024, 1024], b_out[1024]; H=16 heads, head_dim=64.

Sharding: each core owns 2 heads (128 channels). Per core:
  - QKV projection for its 384 qkv-channels over all 4096 tokens,
    computed transposed (channels on partitions): qkv^T = W_slice @ x^T.
  - Attention with scores kept TRANSPOSED (S^T[k, q]) so exp(S^T) feeds
    the AV matmul directly as the moving operand -- no P transposes.
    Softmax skips the max-subtraction (scores are O(6) here, exp cannot
    overflow), which removes all running-max machinery; the row-sum
    comes free from a ones-column appended to V. Causal structure skips
    above-diagonal blocks and truncates diagonal-band blocks' q-range.
    The per-q 1/rowsum is broadcast across partitions via a tiny DRAM
    round-trip (SBUF-source DMAs cannot partition-broadcast and the
    gpsimd partition_broadcast op is broken on HW).
  - out_proj partial: ctx^T (d on partitions) @ w_out slice -> full-size
    partial output (bf16); host sums the 8 partials in fp32 (+ b_out).
"""

import numpy as np
import ml_dtypes

import concourse.bass as bass
import concourse.mybir as mybir
import concourse.tile as tile
from concourse import bacc
from concourse.bass_utils import run_bass_kernel_spmd
from concourse.masks import make_identity

# ---- problem dims (hardcoded per contest rules) ----
D = 1024
H = 16
HD = 64
B = 2
T = 2048
NT = B * T            # 4096 tokens total
NCORES = 8
HPC = H // NCORES     # 2 heads per core
CH = HPC * HD         # 128 channels per core
QB = 512              # q-block (matmul moving dim)
KB = 128              # k-block (psum partition dim)
NQB = T // QB         # 4 q-blocks per batch
NTB = NT // QB        # 8 token blocks for projection
NKC = D // 128        # 8 contraction chunks for projections

# ---- dtype config ----
MM_KIND = "bf16"      # "bf16" | "f32r" | "f32"


def _dts(kind):
    if kind == "bf16":
        return dict(store=mybir.dt.bfloat16, np=ml_dtypes.bfloat16, mmcast=None)
    if kind == "f32r":
        return dict(store=mybir.dt.float32, np=np.float32, mmcast=mybir.dt.float32r)
    if kind == "f32":
        return dict(store=mybir.dt.float32, np=np.float32, mmcast=None)
    raise ValueError(kind)


def _mm(ap, cast):
    return ap.bitcast(cast) if cast is not None else ap


def build_kernel(kind=MM_KIND):
    cfg = _dts(kind)
    sdt = cfg["store"]
    cast = cfg["mmcast"]

    nc = bacc.Bacc("TRN2", target_bir_lowering=False, debug=False,
                   enable_asserts=False, num_devices=NCORES)

    xT_d = nc.dram_tensor("xT", [D, NT], sdt, kind="ExternalInput")
    wqkvT_d = nc.dram_tensor("wqkvT", [D, 3 * CH], sdt, kind="ExternalInput")
    bqkv_d = nc.dram_tensor("bqkv", [3, CH], mybir.dt.float32, kind="ExternalInput")
    woutT_d = nc.dram_tensor("woutT", [CH, D], sdt, kind="ExternalInput")
    outp_d = nc.dram_tensor("outp", [NT, D], mybir.dt.bfloat16, kind="ExternalOutput")
    # scratch for the 1/rowsum partition-broadcast round-trip
    rs_d = nc.dram_tensor("rs_scratch", [B, NQB, HPC * QB], mybir.dt.float32)

    xT = xT_d.ap().rearrange("(kc p) t -> p kc t", p=128)
    wqkvT = wqkvT_d.ap().rearrange("(kc p) j -> p kc j", p=128)
    bqkv = bqkv_d.ap().rearrange("m p -> p m")
    woutT = woutT_d.ap()
    outp = outp_d.ap()

    with tile.TileContext(nc, trace_sim=False) as tc:
        with (
            tc.tile_pool(name="consts", bufs=1) as consts,
            tc.tile_pool(name="px", bufs=2) as px,
            tc.tile_pool(name="pqkv", bufs=1) as pqkv,
            tc.tile_pool(name="pv", bufs=1) as pv,
            tc.tile_pool(name="pctx", bufs=1) as pctx,
            tc.tile_pool(name="pexp", bufs=6) as pexp,
            tc.tile_pool(name="pnorm", bufs=4) as pnorm,
            tc.tile_pool(name="pstage", bufs=2) as pstage,
            tc.tile_pool(name="pout", bufs=8) as pout,
            tc.tile_pool(name="ps_misc", bufs=2, space="PSUM") as ps_misc,
            tc.tile_pool(name="ps_s", bufs=2, space="PSUM") as ps_s,
            tc.tile_pool(name="ps_c", bufs=2, space="PSUM") as ps_c,
        ):
            # ---- constants ----
            ident = consts.tile([128, 128], sdt, tag="ident")
            make_identity(nc, ident)
            # tri[j, :, i] = 1 where j <= i (keep), else 0; multiplied into
            # exp(S^T) on diagonal blocks (both heads in one op).
            tri = consts.tile([128, HPC, 128], sdt, tag="tri")
            nc.gpsimd.memset(tri, 1.0)
            nc.gpsimd.affine_select(
                out=tri, in_=tri, compare_op=mybir.AluOpType.is_ge,
                fill=0.0, base=0, pattern=[[0, HPC], [1, 128]],
                channel_multiplier=-1,
            )
            ones = consts.tile([128, 64], mybir.dt.float32, tag="ones")
            nc.vector.memset(ones, 1.0)
            w_sb = consts.tile([128, NKC, 3 * CH], sdt, tag="w")
            b_sb = consts.tile([128, 3], mybir.dt.float32, tag="b")
            wo_sb = consts.tile([128, D], sdt, tag="wo")

            # persistent activations
            qkvT = pqkv.tile([128, 3, NT], sdt, tag="qkvT")  # [qT|kT|vT] x tokens
            v_sb = pv.tile([128, B, HPC, T // KB, 65], sdt, tag="v")
            nc.vector.memset(v_sb[:, :, :, :, 64:65], 1.0)   # ones column
            ctxT = pctx.tile([128, B, T], sdt, tag="ctxT")

            def emit_qkv(tb):
                """QKV^T projection + V transposes for one 512-token block."""
                xt = px.tile([128, NKC, QB], sdt, tag="x", name=f"xt{tb}")
                for kc in range(NKC):
                    if tb == 0:
                        nc.sync.dma_start(out=w_sb[:, kc, :],
                                          in_=wqkvT[:, kc, :])
                    nc.sync.dma_start(
                        out=xt[:, kc, :],
                        in_=xT[:, kc, tb * QB:(tb + 1) * QB])
                if tb == 0:
                    nc.sync.dma_start(out=b_sb, in_=bqkv)
                    nc.sync.dma_start(out=wo_sb, in_=woutT)
                for m in range(3):
                    ps = ps_misc.tile([128, QB], mybir.dt.float32, tag="m",
                                      name=f"qkvps{tb}_{m}")
                    for kc in range(NKC):
                        nc.tensor.matmul(
                            ps,
                            _mm(w_sb[:, kc, m * CH:(m + 1) * CH], cast),
                            _mm(xt[:, kc, :], cast),
                            start=(kc == 0), stop=(kc == NKC - 1),
                        )
                    nc.vector.tensor_scalar_add(
                        qkvT[:, m, tb * QB:(tb + 1) * QB], ps, b_sb[:, m:m + 1])
                for j in range(QB // KB):
                    tj = tb * (QB // KB) + j          # global 128-token block
                    bb, jb = divmod(tj, T // KB)
                    pst = ps_misc.tile([128, 128], sdt, tag="m",
                                       name=f"vtp{tj}")
                    nc.tensor.transpose(
                        pst, qkvT[:, 2, tj * KB:(tj + 1) * KB], ident)
                    for hl in range(HPC):
                        nc.vector.tensor_copy(
                            v_sb[:, bb, hl, jb, 0:HD],
                            pst[:, hl * HD:(hl + 1) * HD])

            _ncopy = [0]

            _otiles = {}

            def out_proj_units(b, qb):
                """out_proj partial for one q-block's token range, as
                single-matmul closures for fine-grain interleaving."""
                return [lambda b=b, tt=tt, e=e: emit_out_tile(b, tt, e)
                        for tt in range(qb * (QB // 128), (qb + 1) * (QB // 128))
                        for e in range(D // QB)]

            def emit_out_tile(b, tt, e):
                if e == 0:
                    _otiles[(b, tt)] = pout.tile(
                        [128, D // QB, QB], mybir.dt.bfloat16, tag="o",
                        name=f"ot{b}_{tt}")
                ot = _otiles[(b, tt)]
                po = ps_misc.tile([128, QB], mybir.dt.float32, tag="m",
                                  name=f"po{b}_{tt}_{e}")
                nc.tensor.matmul(
                    po,
                    _mm(ctxT[:, b, tt * 128:(tt + 1) * 128], cast),
                    _mm(wo_sb[:, e * QB:(e + 1) * QB], cast),
                    start=True, stop=True,
                )
                if _ncopy[0] % 3 == 0:
                    nc.scalar.copy(ot[:, e, :], po)
                else:
                    nc.vector.tensor_copy(ot[:, e, :], po)
                _ncopy[0] += 1
                if e == D // QB - 1:
                    nc.sync.dma_start(
                        out=outp[b * T + tt * 128:b * T + (tt + 1) * 128, :],
                        in_=ot)

            def emit_attention(b, qb, fillers, terminal=False):
                nkb = (QB // KB) * (qb + 1)
                psc = [ps_c.tile([65, QB], mybir.dt.float32, tag="c",
                                 name=f"psc{b}_{qb}_{hl}")
                       for hl in range(HPC)]
                for kb in range(nkb):
                    r = kb - (QB // KB) * qb          # >= 0: diagonal band
                    c0 = KB * max(r, 0)
                    qlen = QB - c0
                    k0 = b * T + kb * KB
                    qs = b * T + qb * QB + c0
                    ps = ps_s.tile([128, HPC, QB], mybir.dt.float32, tag="s",
                                   name=f"sc{b}_{qb}_{kb}")
                    for hl in range(HPC):
                        hp = hl * HD
                        nc.tensor.matmul(
                            ps[:, hl, 0:qlen],
                            _mm(qkvT[hp:hp + HD, 1, k0:k0 + KB], cast),
                            _mm(qkvT[hp:hp + HD, 0, qs:qs + qlen], cast),
                            start=True, stop=True,
                        )
                    es = pexp.tile([128, HPC, qlen], sdt, tag="e",
                                   name=f"es{b}_{qb}_{kb}")
                    nc.scalar.activation(
                        es, ps[:, :, 0:qlen],
                        mybir.ActivationFunctionType.Exp, scale=0.125)
                    if r >= 0:
                        nc.vector.tensor_mul(
                            es[:, :, 0:KB], es[:, :, 0:KB], tri)
                    for hl in range(HPC):
                        nc.tensor.matmul(
                            psc[hl][0:65, c0:QB],
                            _mm(v_sb[:, b, hl, kb, :], cast),
                            _mm(es[:, hl, :], cast),
                            start=(kb == 0), stop=(kb == nkb - 1),
                        )
                    if kb % 2 == 1 and fillers:
                        fillers.pop(0)()
                # normalize: ctx^T[:, q] * (1/rowsum[q]); the reciprocal row
                # is partition-broadcast via a DRAM round-trip (engine-free;
                # latency hidden by the lagged out_proj), except the terminal
                # section which uses a PE ones-outer-product (short chain).
                rstage = pnorm.tile([65, HPC * QB], mybir.dt.float32, tag="r",
                                    name=f"rst{b}_{qb}")
                for hl in range(HPC):
                    nc.vector.reciprocal(
                        rstage[64:65, hl * QB:(hl + 1) * QB],
                        psc[hl][64:65, :])
                bc = pnorm.tile([64, HPC * QB], mybir.dt.float32, tag="n",
                                name=f"bc{b}_{qb}")
                if terminal:
                    bc_ps = ps_s.tile([64, HPC * QB], mybir.dt.float32,
                                      tag="s", name=f"bcp{b}_{qb}")
                    for hl in range(HPC):
                        nc.tensor.matmul(
                            bc_ps[:, hl * QB:(hl + 1) * QB],
                            ones[64:65, :],
                            rstage[64:65, hl * QB:(hl + 1) * QB],
                            start=True, stop=True)
                    nc.vector.tensor_copy(bc, bc_ps)
                else:
                    rs_slot = rs_d.ap()[b, qb, :]
                    nc.sync.dma_start(out=rs_slot, in_=rstage[64:65, :])
                    bc_src = bass.AP(
                        tensor=rs_slot.tensor, offset=rs_slot.offset,
                        ap=[[0, 64]] + rs_slot.ap)
                    nc.sync.dma_start(out=bc, in_=bc_src)
                qcols = slice(qb * QB, (qb + 1) * QB)
                for hl in reversed(range(HPC)):
                    if hl == 0:
                        nc.vector.tensor_mul(
                            ctxT[0:HD, b, qcols], psc[hl][0:HD, :],
                            bc[0:HD, 0:QB])
                    else:
                        st = pstage.tile([HD, QB], sdt, tag="st",
                                         name=f"st{b}_{qb}")
                        nc.vector.tensor_mul(st, psc[hl][0:HD, :],
                                             bc[0:HD, QB:2 * QB])
                        nc.sync.dma_start(
                            out=ctxT[HD:2 * HD, b, qcols], in_=st)

            # driver: interleave projection token-blocks with attention
            # q-blocks; out_proj tiles trail their q-block's normalize by
            # one attention section and are woven into the next section's
            # kb loop so the PE never idles on a normalize chain.
            fillers = []
            pending = []
            for b in range(B):
                for qb in range(NQB):
                    emit_qkv(b * NQB + qb)
                    terminal = (b == B - 1 and qb == NQB - 1)
                    emit_attention(b, qb, fillers, terminal=terminal)
                    if pending:
                        fillers.extend(out_proj_units(*pending.pop(0)))
                    pending.append((b, qb))
            for chunk in pending:
                fillers.extend(out_proj_units(*chunk))
            for f in fillers:
                f()

    nc.compile()
    return nc


def prep_inputs(x, w_in, b_in, w_out, kind=MM_KIND):
    """Host-side sharding: returns per-core input maps."""
    npdt = _dts(kind)["np"]
    xT = np.ascontiguousarray(x.reshape(NT, D).T).astype(npdt)
    in_maps = []
    for c in range(NCORES):
        rows = []
        brows = []
        for part in range(3):  # q, k, v
            lo = part * D + c * CH
            rows.append(w_in[lo:lo + CH, :])
            brows.append(b_in[lo:lo + CH])
        wqkvT = np.ascontiguousarray(np.concatenate(rows, 0).T).astype(npdt)
        bqkv = np.stack(brows, 0).astype(np.float32)          # [3, 128]
        woutT = np.ascontiguousarray(w_out[:, c * CH:(c + 1) * CH].T).astype(npdt)
        in_maps.append({"xT": xT, "wqkvT": wqkvT, "bqkv": bqkv, "woutT": woutT})
    return in_maps


_NC_CACHE = {}


def kernel(x, w_in, b_in, w_out, b_out):
    x = np.asarray(x, dtype=np.float32)
    w_in = np.asarray(w_in, dtype=np.float32)
    b_in = np.asarray(b_in, dtype=np.float32)
    w_out = np.asarray(w_out, dtype=np.float32)
    b_out = np.asarray(b_out, dtype=np.float32)

    if MM_KIND not in _NC_CACHE:
        _NC_CACHE[MM_KIND] = build_kernel(MM_KIND)
    nc = _NC_CACHE[MM_KIND]

    in_maps = prep_inputs(x, w_in, b_in, w_out, MM_KIND)
    res = run_bass_kernel_spmd(nc, in_maps, core_ids=list(range(NCORES)))
    out = np.zeros((NT, D), dtype=np.float32)
    for r in res.results:
        out += r["outp"].astype(np.float32)
    out += b_out[None, :]
    return out.reshape(B, T, D).astype(np.float32)
